# revision 5
# baseline (speedup 1.0000x reference)
"""Trainium kernel for nn_PesudoLabelGenerator_v9_8 (retrieval_knn).

Strategy (M sharded over 8 cores, Mc=1250 padded to 1280):
  Phase 1: per core, compute masked kernel K.T slab [m,n] = exp((q+e1-2)/.03)*[d<100]
           via two PE matmuls (d: 5-contract folded-norm trick, q: 96-contract
           normalized features), spill slab to DRAM, partial rowsum over m via
           ones-matmul -> AllReduce [1,6144].
  Phase 2: b = 1/(rowsum+1e-12) (scale-invariant for argmax), broadcast via PE,
           Z = K.T * b, per-column argmax over n via DVE max/max_index.
  Phase 3: gather T_E rows by sm_index (indirect DMA), PE-transpose -> lhsT,
           single 104-contract matmul gives exponent E of dist_matric directly,
           ACT exp with per-partition colmask bias, contract with [onehot|1]
           -> [21, n] partials -> AllReduce.
  Host: everything O(N+M): align, normalization, softmax, scatter, trust select.
"""

import numpy as np

N, M, C, OUT, LG_DEF = 6000, 10000, 96, 20, 2000
NCORES = 8
MC = M // NCORES      # 1250
MP = 1280             # padded per-core M
MT = MP // 128        # 10 m-tiles
NP = 6144             # padded N
NCH = NP // 512       # 12 n-chunks
KE = 104              # E-matmul contract (3 coords + 96 feat + 2 norm rows + 3 zero pad)

_CACHE = {}


def _ran_mask():
    if "ran" not in _CACHE:
        import jax

        with jax.default_device(jax.devices("cpu")[0]):
            _CACHE["ran"] = np.asarray(
                jax.random.randint(jax.random.key(42), (M,), 0, 20) == 1
            )
    return _CACHE["ran"]


def _build_nc():
    import concourse.bass as bass
    import concourse.bacc as bacc
    import concourse.mybir as mybir
    from concourse import masks
    from concourse.tile import TileContext

    f32 = mybir.dt.float32
    u32 = mybir.dt.uint32
    Alu = mybir.AluOpType
    Act = mybir.ActivationFunctionType

    nc = bacc.Bacc(
        "TRN2", target_bir_lowering=False, debug=False, num_devices=NCORES
    )

    # per-core inputs
    lhstd = nc.dram_tensor("lhstd", [5, MP], f32, kind="ExternalInput")
    lhstq = nc.dram_tensor("lhstq", [C, MP], f32, kind="ExternalInput")
    oh = nc.dram_tensor("oh", [MT, 128, OUT + 1], f32, kind="ExternalInput")
    cmb = nc.dram_tensor("cmb", [MT, 128], f32, kind="ExternalInput")
    # replicated inputs
    rhsd = nc.dram_tensor("rhsd", [5, NP], f32, kind="ExternalInput")
    rhsq = nc.dram_tensor("rhsq", [C, NP], f32, kind="ExternalInput")
    rhse = nc.dram_tensor("rhse", [KE, NP], f32, kind="ExternalInput")
    te = nc.dram_tensor("te", [N, KE], f32, kind="ExternalInput")
    # outputs
    sm_out = nc.dram_tensor("sm_out", [MT, 128], u32, kind="ExternalOutput")
    uw_out = nc.dram_tensor("uw_out", [OUT + 1, NP], f32, kind="ExternalOutput")

    with TileContext(nc) as tc:
        with (
            tc.tile_pool(name="dram", bufs=1, space="DRAM") as dpool,
            tc.tile_pool(name="persist", bufs=1) as pp,
        ):
            rbuf = dpool.tile([MT, 128, NP], f32)
            rs_in = dpool.tile([1, NP], f32)
            rs_ar = dpool.tile([1, NP], f32, addr_space="Shared")
            uw_in = dpool.tile([OUT + 1, NP], f32)
            uw_ar = dpool.tile([OUT + 1, NP], f32, addr_space="Shared")

            idx_all = pp.tile([128, MT], u32)   # fixed argmax per m-tile
            ones_col = pp.tile([128, 1], f32)   # rowsum lhsT
            nc.vector.memset(ones_col[:], 1.0)
            ones_row = pp.tile([1, 128], f32)   # bcast lhsT
            nc.vector.memset(ones_row[:], 1.0)
            kbias = pp.tile([128, 1], f32)      # -2/0.03 bias for K exp
            nc.vector.memset(kbias[:], float(-2.0 / 0.03))

            # ---------------- Phase 1 ----------------
            with (
                tc.tile_pool(name="p1c", bufs=1) as cp,
                tc.tile_pool(name="p1w", bufs=3) as wp,
                tc.tile_pool(name="p1ps", bufs=2, space="PSUM") as psp,
                tc.tile_pool(name="p1rs", bufs=2, space="PSUM") as psr,
            ):
                lhstd_sb = cp.tile([5, MP], f32, tag="lhstd")
                nc.sync.dma_start(lhstd_sb[:], lhstd[:])
                lhstq_sb = cp.tile([C, MP], f32, tag="lhstq")
                nc.sync.dma_start(lhstq_sb[:], lhstq[:])
                rhsd_sb = cp.tile([5, NP], f32, tag="rhsd")
                nc.sync.dma_start(rhsd_sb[:], rhsd[:])
                rhsq_sb = cp.tile([C, NP], f32, tag="rhsq")
                nc.sync.dma_start(rhsq_sb[:], rhsq[:])
                rs_sb = cp.tile([1, NP], f32, tag="rs")

                for j in range(NCH):
                    njs = slice(j * 512, (j + 1) * 512)
                    rs_ps = psr.tile([1, 512], f32, tag="rsps")
                    for i in range(MT):
                        mis = slice(i * 128, (i + 1) * 128)
                        d_ps = psp.tile([128, 512], f32, tag="dps")
                        nc.tensor.matmul(
                            d_ps[:], lhstd_sb[:, mis], rhsd_sb[:, njs],
                            start=True, stop=True,
                        )
                        q_ps = psp.tile([128, 512], f32, tag="qps")
                        nc.tensor.matmul(
                            q_ps[:], lhstq_sb[:, mis], rhsq_sb[:, njs],
                            start=True, stop=True,
                        )
                        e1 = wp.tile([128, 512], f32, tag="e1")
                        nc.scalar.activation(e1[:], d_ps[:], Act.Exp, scale=-2.0)
                        pen = wp.tile([128, 512], f32, tag="pen")
                        nc.vector.tensor_scalar(
                            pen[:], d_ps[:], 100.0, -30000.0, Alu.is_ge, Alu.mult
                        )
                        x = wp.tile([128, 512], f32, tag="x")
                        nc.vector.tensor_tensor(x[:], q_ps[:], e1[:], Alu.add)
                        x2 = wp.tile([128, 512], f32, tag="x2")
                        nc.vector.tensor_tensor(x2[:], x[:], pen[:], Alu.add)
                        r = wp.tile([128, 512], f32, tag="r")
                        nc.scalar.activation(
                            r[:], x2[:], Act.Exp,
                            scale=float(1.0 / 0.03), bias=kbias[:, 0:1],
                        )
                        nc.tensor.matmul(
                            rs_ps[:], ones_col[:], r[:],
                            start=(i == 0), stop=(i == MT - 1),
                        )
                        nc.sync.dma_start(rbuf[i, :, njs], r[:])
                    nc.scalar.copy(rs_sb[:, njs], rs_ps[:])

                nc.sync.dma_start(rs_in[:], rs_sb[:])

            nc.gpsimd.collective_compute(
                "AllReduce",
                mybir.AluOpType.add,
                ins=[rs_in[:].opt()],
                outs=[rs_ar[:].opt()],
                replica_groups=[list(range(NCORES))],
            )

            # ---------------- Phase 2 ----------------
            with (
                tc.tile_pool(name="p2c", bufs=1) as cp2,
                tc.tile_pool(name="p2b", bufs=2) as bp2,
                tc.tile_pool(name="p2w", bufs=2) as wp2,
                tc.tile_pool(name="p2ps", bufs=2, space="PSUM") as psp2,
            ):
                rsb = bp2.tile([1, NP], f32, tag="rsb")
                nc.sync.dma_start(rsb[:], rs_ar[:])
                binv = bp2.tile([1, NP], f32, tag="rsb")
                nc.vector.tensor_scalar(binv[:], rsb[:], 1e-12, None, Alu.add)
                nc.vector.reciprocal(binv[:], binv[:])
                bb = cp2.tile([128, NP], f32, tag="bb")
                for j in range(NCH):
                    njs = slice(j * 512, (j + 1) * 512)
                    bb_ps = psp2.tile([128, 512], f32, tag="bbps")
                    nc.tensor.matmul(
                        bb_ps[:], ones_row[:], binv[:, njs], start=True, stop=True
                    )
                    nc.scalar.copy(bb[:, njs], bb_ps[:])

                for i in range(MT):
                    rt = wp2.tile([128, NP], f32, tag="rt")
                    nc.sync.dma_start(rt[:], rbuf[i])
                    nc.vector.tensor_tensor(rt[:], rt[:], bb[:], Alu.mult)
                    mx8 = wp2.tile([128, 8], f32, tag="mx8")
                    nc.vector.max(mx8[:], rt[:])
                    ix8 = wp2.tile([128, 8], u32, tag="ix8")
                    nc.vector.max_index(ix8[:], mx8[:], rt[:])
                    # zero out rows whose max is 0 (all-masked columns): jax argmax -> 0
                    msk = wp2.tile([128, 1], f32, tag="msk")
                    nc.vector.tensor_scalar(msk[:], mx8[:, 0:1], 0.0, None, Alu.is_gt)
                    ixf = wp2.tile([128, 1], f32, tag="ixf")
                    nc.scalar.copy(ixf[:], ix8[:, 0:1])
                    nc.vector.tensor_scalar(ixf[:], ixf[:], msk[:, 0:1], None, Alu.mult)
                    nc.scalar.copy(idx_all[:, i : i + 1], ixf[:])
                    nc.sync.dma_start(sm_out[i, :], idx_all[:, i])

            # ---------------- Phase 3 ----------------
            with (
                tc.tile_pool(name="p3c", bufs=1) as cp3,
                tc.tile_pool(name="p3w", bufs=3) as wp3,
                tc.tile_pool(name="p3ps", bufs=2, space="PSUM") as psp3,
                tc.tile_pool(name="p3uw", bufs=2, space="PSUM") as psu3,
            ):
                rhse_sb = cp3.tile([KE, NP], f32, tag="rhse")
                nc.sync.dma_start(rhse_sb[:], rhse[:])
                id_sb = cp3.tile([128, 128], f32, tag="ident")
                masks.make_identity(nc, id_sb[:])
                lhe = cp3.tile([KE, MP], f32, tag="lhe")
                oh_sb = cp3.tile([128, MT * (OUT + 1)], f32, tag="ohsb")
                cmb_sb = cp3.tile([128, MT], f32, tag="cmbsb")
                uw_sb = cp3.tile([OUT + 1, NP], f32, tag="uwsb")

                import concourse.bass as bass_mod

                for i in range(MT):
                    nc.sync.dma_start(
                        oh_sb[:, i * (OUT + 1) : (i + 1) * (OUT + 1)], oh[i]
                    )
                    nc.sync.dma_start(cmb_sb[:, i : i + 1], cmb[i, :])
                    g = wp3.tile([128, KE], f32, tag="gat")
                    nc.gpsimd.indirect_dma_start(
                        out=g[:],
                        out_offset=None,
                        in_=te[:],
                        in_offset=bass_mod.IndirectOffsetOnAxis(
                            ap=idx_all[:, i : i + 1], axis=0
                        ),
                    )
                    tr_ps = psp3.tile([128, 128], f32, tag="trps")
                    nc.tensor.transpose(tr_ps[:KE, :], g[:], id_sb[:])
                    nc.scalar.copy(lhe[:, i * 128 : (i + 1) * 128], tr_ps[:KE, :])

                for j in range(NCH):
                    njs = slice(j * 512, (j + 1) * 512)
                    uw_ps = psu3.tile([OUT + 1, 512], f32, tag="uwps")
                    for i in range(MT):
                        e_ps = psp3.tile([128, 512], f32, tag="eps")
                        nc.tensor.matmul(
                            e_ps[:], lhe[:, i * 128 : (i + 1) * 128],
                            rhse_sb[:, njs], start=True, stop=True,
                        )
                        dist = wp3.tile([128, 512], f32, tag="dist")
                        nc.scalar.activation(
                            dist[:], e_ps[:], Act.Exp,
                            scale=1.0, bias=cmb_sb[:, i : i + 1],
                        )
                        nc.tensor.matmul(
                            uw_ps[:],
                            oh_sb[:, i * (OUT + 1) : (i + 1) * (OUT + 1)],
                            dist[:],
                            start=(i == 0), stop=(i == MT - 1),
                        )
                    nc.scalar.copy(uw_sb[:, njs], uw_ps[:])

                nc.sync.dma_start(uw_in[:], uw_sb[:])

            nc.gpsimd.collective_compute(
                "AllReduce",
                mybir.AluOpType.add,
                ins=[uw_in[:].opt()],
                outs=[uw_ar[:].opt()],
                replica_groups=[list(range(NCORES))],
            )
            nc.sync.dma_start(uw_out[:], uw_ar[:])

    nc.finalize()
    return nc


def _prep_inputs(sur_sv_feature, sur_sv_coords, sur_sv_gt, sv_prob,
                 mean_features, ori_coords, posses, lg):
    f = np.float32
    # align (mirror reference in fp32 LAPACK/BLAS)
    p0 = np.asarray(posses[0], f)
    p1 = np.asarray(posses[1], f)
    diff = np.linalg.inv(p1) @ p0
    h = np.concatenate(
        [np.asarray(sur_sv_coords, f), np.ones((M, 1), f)], axis=1
    )
    al = (h @ diff.T)[:, :3].astype(f)

    mf = np.asarray(mean_features, f)
    sf = np.asarray(sur_sv_feature, f)
    ori = np.asarray(ori_coords, f)
    gt = np.asarray(sur_sv_gt, f)

    w1 = np.maximum(np.linalg.norm(mf, axis=1), 1e-12).astype(f)
    w2 = np.maximum(np.linalg.norm(sf, axis=1), 1e-12).astype(f)
    mfn = mf / w1[:, None]
    sfn = sf / w2[:, None]

    nrm_al = (al * al).sum(1).astype(f)       # [M]
    nrm_ori = (ori * ori).sum(1).astype(f)    # [N]
    nrm_mf = (mf * mf).sum(1).astype(f)       # [N]

    # phase-1 rhs (replicated): d = -2*al.oriT + nrm_al + nrm_ori
    rhsd = np.zeros((5, NP), f)
    rhsd[:3, :N] = ori.T
    rhsd[3, :] = 1.0
    rhsd[4, :N] = nrm_ori
    rhsd[4, N:] = 1e9
    rhsq = np.zeros((C, NP), f)
    rhsq[:, :N] = mfn.T

    # phase-3: E[m,n] = 16*o_sm.o_n + (1/0.09)*g_sm.g_n + A_sm + A_n
    # with A_k = -8*|o_k|^2 - (0.5/0.09)*|g_k|^2 ; dist = exp(E)*colmask
    cf = np.float32(0.5 / 0.09)
    A = (-8.0 * nrm_ori - cf * nrm_mf).astype(f)    # [N]
    te_tab = np.zeros((N, KE), f)
    te_tab[:, :3] = 16.0 * ori
    te_tab[:, 3 : 3 + C] = (2.0 * cf) * mf
    te_tab[:, 3 + C] = A
    te_tab[:, 4 + C] = 1.0
    rhse = np.zeros((KE, NP), f)
    rhse[:3, :N] = ori.T
    rhse[3 : 3 + C, :N] = mf.T
    rhse[3 + C, :N] = 1.0
    rhse[4 + C, :N] = A
    rhse[4 + C, N:] = -1e30   # padded n -> dist 0

    # upd/colmask (host, no sm dependency)
    lg = int(lg)
    upd = np.argmax(gt, axis=1) < 9
    upd[:lg] = True
    upd |= _ran_mask()

    in_maps = []
    for c in range(NCORES):
        sl = slice(c * MC, (c + 1) * MC)
        lhstd = np.zeros((5, MP), f)
        lhstd[:3, :MC] = -2.0 * al[sl].T
        lhstd[3, :MC] = nrm_al[sl]
        lhstd[3, MC:] = 1e9
        lhstd[4, :] = 1.0
        lhstq = np.zeros((C, MP), f)
        lhstq[:, :MC] = sfn[sl].T
        oh = np.zeros((MT, 128, OUT + 1), f)
        ohc = np.zeros((MP, OUT + 1), f)
        ohc[:MC, :OUT] = gt[sl]
        ohc[:MC, OUT] = 1.0
        oh[:] = ohc.reshape(MT, 128, OUT + 1)
        cmbv = np.full((MP,), -1e30, f)
        cmbv[:MC] = np.where(upd[sl], 0.0, -1e30).astype(f)
        in_maps.append(
            dict(
                lhstd=lhstd, lhstq=lhstq, oh=oh,
                cmb=cmbv.reshape(MT, 128),
                rhsd=rhsd, rhsq=rhsq, rhse=rhse, te=te_tab,
            )
        )
    return in_maps


def _run_device(in_maps, trace=False):
    from concourse.bass_utils import run_bass_kernel_spmd

    if "nc" not in _CACHE:
        _CACHE["nc"] = _build_nc()
    res = run_bass_kernel_spmd(
        _CACHE["nc"], in_maps, core_ids=list(range(NCORES)), trace=trace
    )
    return res


def _postprocess(results, sur_sv_gt, sv_prob, mean_features, ori_coords, lg):
    f = np.float32
    lg = int(lg)
    gt = np.asarray(sur_sv_gt, f)
    sm_index = np.concatenate(
        [results[c]["sm_out"].reshape(MP)[:MC] for c in range(NCORES)]
    ).astype(np.int32)
    uw_acc = results[0]["uw_out"]
    num = uw_acc[:OUT, :N].T
    den = uw_acc[OUT, :N][:, None]
    uw = (num / (den + 1e-16)).astype(f)

    sp = np.asarray(sv_prob, f)
    e = np.exp(sp - sp.max(axis=1, keepdims=True))
    sv_prob_sm = (e / e.sum(axis=1, keepdims=True)).astype(f)
    update = (0.5 * sv_prob_sm + 0.5 * uw).astype(f)

    tmpx = np.argmax(gt[:lg], axis=1)
    vals = update[sm_index[:lg], tmpx]
    tmp1 = vals > 0.1
    tmp = tmp1 if tmp1.any() else (vals > 0.0)
    idxs = np.where(tmp, sm_index[:lg], N)
    supd = update.copy()
    keep = idxs < N
    supd[idxs[keep]] = gt[:lg][keep]

    mx = supd.max(axis=1)
    m1 = mx >= 0.9
    tm = m1 if m1.any() else (mx >= 0.85)
    trust_svm = np.nonzero(tm)[0].astype(np.int32)
    pre = np.zeros((N, OUT), f)
    pre[np.arange(N), np.argmax(supd, axis=1)] = 1.0

    mf = np.asarray(mean_features, f)
    ori = np.asarray(ori_coords, f)
    return (
        trust_svm,
        mf[trust_svm],
        ori[trust_svm],
        pre[trust_svm],
        uw,
        supd,
        sm_index,
    )


def kernel(sur_sv_feature, sur_sv_coords, sur_sv_gt, sv_prob, mean_features,
           ori_coords, posses, lg, idx=1, rev=1, _trace=False):
    in_maps = _prep_inputs(
        sur_sv_feature, sur_sv_coords, sur_sv_gt, sv_prob,
        mean_features, ori_coords, posses, lg,
    )
    res = _run_device(in_maps, trace=_trace)
    out = _postprocess(
        res.results, sur_sv_gt, sv_prob, mean_features, ori_coords, lg
    )
    if _trace:
        return out, res
    return out
